# revision 26
# baseline (speedup 1.0000x reference)
"""Multi-head attention kernel for Trainium2, sharded over 8 NeuronCores.

Problem: B=4, S=2048, D=256, H=8 dense transformer attention block
(per-head K/V/Q Linear projections + dot-product attention + output Linear).

Sharding: core = (batch b, head-group g); core 2*b+g handles batch b and
heads [4g, 4g+4). Each core computes its heads' contribution to the final
output Linear (Wo rows h::H belong to head h); the host sums the two
partial outputs per batch and adds the (host-folded) bias.

Algebraic folds (host-side, exact up to fp32 rounding):
  - scores = (kWk+bk)(qWq'+bq')^T with Wq'=Wq/16, bq'=bq/16 expands to
      k M q^T + ku[m] + (per-query terms)
    where M = Wk Wq'^T and ku = k (Wk bq'). The per-query terms are
    constant along the softmax axis (keys) and cancel; ku becomes the Exp
    activation's per-partition bias. So the kernel needs no Q projection
    and no K/Q bias adds at all.
  - AV+output: w^T (v Wv + bv) Wo_h = w^T (v W2) + bv Wo_h with
    W2 = Wv Wo_h, so AV directly produces output-space values (transposed;
    host transposes back) and bo' = bo + sum_h bv[h] Wo_h is added on host.
  - k/v/q are transposed to [D, S] on host so no on-chip transposes occur.

fp8 acceleration (vs the bf16/f32r baseline):
  - scores, AV, and the softmax denominator all run as fp8e4m3
    DoubleRow matmuls: K=256 packed per instruction at 0.5 cycles/row,
    2-4x the bf16 MAC rate. tT (projected k) is pre-scaled by 128 (folded
    into wm on host) so its fp8 quantization stays in the normal range;
    the Exp activation descales via its scale operand. q is quantized to
    fp8 on the host. V2 is pre-scaled by 16 (folded into W2); the
    denominator's ones-matmul uses 16.0 so the scale cancels exactly in
    the normalize step.
  - The softmax denominator is a DoubleRow matmul with a constant
    "ones" (=16) stationary tile, which also broadcasts the per-query sum
    across all 128 partitions for free (replaces the DVE reduce tree).
  - ACT runs nothing but 1024-wide Exp activations (two per key-tile,
    each spanning 2 query blocks of one key-tile so a single
    per-partition ku bias applies; [P,1024] PSUM tiles double-buffered
    across 4 banks keep ACT gapless); ACT is the ~133us/core roofline
    engine for this regime (16.8M exps at 128 lanes x 1.2GHz).
  - PE work is software-pipelined at half-head granularity: during a
    16-slot score half-phase the PE also runs AV+denominator of the
    previous half and the projections of the next head; the final
    half's AV work carries into the next repeat iteration so back-to-
    back iterations overlap to the ACT roofline.
"""

import numpy as np
from collections import deque
from contextlib import ExitStack

import ml_dtypes

import concourse.bacc as bacc
import concourse.bass as bass
import concourse.tile as tile
from concourse import mybir
from concourse.bass_utils import run_bass_kernel_spmd

B, S, D, H = 4, 2048, 256, 8
P = 128
DC = D // P            # 2 contraction halves of d'
HPC = H // 2           # 4 heads per core
QB = 512               # query-block width (one PSUM bank)
NQB = S // QB          # 4 query blocks
MT = S // P            # 16 key tiles
MP = MT // 2           # 8 key-tile pairs (DoubleRow K=256)
HS = S // 2
F32 = mybir.dt.float32
F32R = mybir.dt.float32r
FP8 = mybir.dt.float8e4
EXP = mybir.ActivationFunctionType.Exp
COPY = mybir.ActivationFunctionType.Copy
DRM = mybir.MatmulPerfMode.DoubleRow
ST = 128.0   # tT pre-scale (host-folds into wm; Exp descales by 1/ST)
SV = 16.0    # V2 pre-scale (host-folds into w2; ones=SV cancels it)


def build_program(repeat=1, nwarm=24):
    nc = bacc.Bacc(None, target_bir_lowering=False)

    ktd = nc.dram_tensor("kt", [D, S], F32R, kind="ExternalInput")
    vtd = nc.dram_tensor("vt", [D, S], F32R, kind="ExternalInput")
    qtd = nc.dram_tensor("qt", [D, S], FP8, kind="ExternalInput")
    wmd = nc.dram_tensor("wm", [HPC, D, D], F32R, kind="ExternalInput")
    w2d = nc.dram_tensor("w2", [HPC, D, D], F32R, kind="ExternalInput")
    kud = nc.dram_tensor("ku", [HPC, P, MT], F32, kind="ExternalInput")
    outd = nc.dram_tensor("out", [D, S], F32, kind="ExternalOutput")

    with ExitStack() as ctx:
        tc = ctx.enter_context(tile.TileContext(nc))
        const = ctx.enter_context(tc.tile_pool(name="const", bufs=1))
        wpool = ctx.enter_context(tc.tile_pool(name="w", bufs=2))
        tpool = ctx.enter_context(tc.tile_pool(name="tT", bufs=2))
        vpool = ctx.enter_context(tc.tile_pool(name="V2", bufs=3))
        epool = ctx.enter_context(tc.tile_pool(name="exp", bufs=2))
        rcpool = ctx.enter_context(tc.tile_pool(name="recip", bufs=2))
        qpool = ctx.enter_context(tc.tile_pool(name="q", bufs=2))
        scpool = ctx.enter_context(tc.tile_pool(name="sc", bufs=4))
        psS = ctx.enter_context(
            tc.tile_pool(name="psS", bufs=2, space=bass.MemorySpace.PSUM))
        psPD = ctx.enter_context(
            tc.tile_pool(name="psPD", bufs=2, space=bass.MemorySpace.PSUM))
        psAV = ctx.enter_context(
            tc.tile_pool(name="psAV", bufs=2, space=bass.MemorySpace.PSUM))

        ones8 = const.tile([P, 2, P], FP8)
        nc.vector.memset(ones8[:], SV)
        dmy = const.tile([P, 1], F32)
        nc.scalar.activation(dmy[:], ones8[:, 0, 0:1], EXP)

        carry = []
        for _rep in range(repeat):
            carry = _build_iteration(
                nc, const, wpool, tpool, vpool, epool, rcpool,
                scpool, qpool, psS, psPD, psAV, ones8,
                ktd, vtd, qtd, wmd, w2d, kud, outd,
                nwarm if _rep == 0 else 0, carry=carry,
                last_rep=(_rep == repeat - 1))
        for u in carry:
            u()

    nc.compile()
    return nc


def _build_iteration(nc, const, wpool, tpool, vpool, epool, rcpool, scpool,
                     qpool, psS, psPD, psAV, ones8,
                     ktd, vtd, qtd, wmd, w2d, kud, outd, nwarm,
                     carry=(), last_rep=True):
    # Warm the PE through the cold p-state window while input DMAs land.
    if nwarm:
        ps_w = psPD.tile([P, QB], F32, tag="psPD")
        for wi in range(nwarm):
            nc.tensor.matmul(ps_w[:, :P], ones8[:, 0, :], ones8[:, 0, :],
                             start=(wi == 0), stop=(wi == nwarm - 1))

    def load_weights(h):
        wm_sb = wpool.tile([P, DC, D], F32R, tag="wm")
        w2_sb = wpool.tile([P, DC, D], F32R, tag="w2")
        ku_sb = wpool.tile([P, MT], F32, tag="ku")
        for dc in range(DC):
            nc.sync.dma_start(wm_sb[:, dc, :], wmd[h, dc * P:(dc + 1) * P, :])
            nc.gpsimd.dma_start(w2_sb[:, dc, :], w2d[h, dc * P:(dc + 1) * P, :])
        nc.sync.dma_start(ku_sb[:], kud[h])
        return wm_sb, w2_sb, ku_sb

    kT = const.tile([P, DC, S], F32R)
    vT = const.tile([P, DC, S], F32R)
    qT8 = qpool.tile([P, DC, S], FP8, tag="qT")
    out_acc = const.tile([P, DC, S], F32)

    # wm/ku for head 0 first on the sync queue, then k in t-proj consumption
    # order; q fp8 (first scores input) ahead of w2 on the gpsimd queue;
    # v on the scalar queue (descriptor issue only -- ACT itself stays
    # exp-only).
    wm0 = wpool.tile([P, DC, D], F32R, tag="wm", name="wm_sb")
    w20 = wpool.tile([P, DC, D], F32R, tag="w2", name="w2_sb")
    ku0 = wpool.tile([P, MT], F32, tag="ku", name="ku_sb")
    nc.sync.dma_start(wm0[:, 0, :], wmd[0, 0:P, :])
    nc.sync.dma_start(kT[:, 0, 0:QB], ktd[0:P, 0:QB])
    nc.sync.dma_start(wm0[:, 1, :], wmd[0, P:2 * P, :])
    nc.sync.dma_start(ku0[:], kud[0])
    nc.gpsimd.dma_start(kT[:, 1, 0:QB], ktd[P:2 * P, 0:QB])
    for mb in range(1, NQB):
        for dc in range(DC):
            sl = slice(mb * QB, (mb + 1) * QB)
            nc.sync.dma_start(kT[:, dc, sl], ktd[dc * P:(dc + 1) * P, sl])
    for hf in range(2):
        for dc in range(DC):
            sl = slice(hf * HS, (hf + 1) * HS)
            nc.gpsimd.dma_start(qT8[:, dc, sl], qtd[dc * P:(dc + 1) * P, sl])
    for dc in range(DC):
        nc.gpsimd.dma_start(w20[:, dc, :], w2d[0, dc * P:(dc + 1) * P, :])
    for hf in range(2):
        for dc in range(DC):
            sl = slice(hf * HS, (hf + 1) * HS)
            nc.sync.dma_start(vT[:, dc, sl], vtd[dc * P:(dc + 1) * P, sl])

    weights = {0: (wm0, w20, ku0)}
    tT8s, V28s, expTs = {}, {}, {}

    def tproj_group(h, et, mb, pool=None, ptag="psPD"):
        def emit():
            wm_sb = weights[h][0]
            tT8 = tT8s[h]
            ps = (pool or psPD).tile([P, QB], F32, tag=ptag)
            for dc in range(DC):
                nc.tensor.matmul(
                    ps[:],
                    wm_sb[:, dc, et * P:(et + 1) * P],
                    kT[:, dc, mb * QB:(mb + 1) * QB],
                    start=(dc == 0), stop=(dc == DC - 1))
            nc.vector.tensor_copy(tT8[:, et, mb * QB:(mb + 1) * QB], ps[:])
        return emit

    def vproj_group(h, mp, pool=None, ptag="psPD"):
        def emit():
            w2_sb = weights[h][1]
            V28 = V28s[h]
            ps = (pool or psPD).tile([P, QB], F32, tag=ptag)
            for half in range(2):
                mt = 2 * mp + half
                for dc in range(DC):
                    nc.tensor.matmul(
                        ps[:, half * D:(half + 1) * D],
                        vT[:, dc, mt * P:(mt + 1) * P],
                        w2_sb[:, dc, :],
                        start=(dc == 0), stop=(dc == DC - 1))
            nc.vector.tensor_copy(V28[:, 2 * mp:2 * mp + 2, :], ps[:])
        return emit

    def proj_units(h):
        tT8s[h] = tpool.tile([P, DC, S], FP8, tag="tT", name="tT8")
        V28s[h] = vpool.tile([P, MT, D], FP8, tag="V2", name="V28")
        units = []
        for mb in range(NQB):
            for et in range(DC):
                units.append(tproj_group(h, et, mb))
        for mp in range(MP):
            units.append(vproj_group(h, mp))
        return units

    def denom_unit(h, nb, cell):
        def emit():
            expT = expTs[h]
            ps = psPD.tile([P, QB], F32, tag="psPD")
            for t in range(MP):
                nc.tensor.matmul(
                    ps[:], ones8[:],
                    expT[:, 2 * t:2 * t + 2, nb * QB:(nb + 1) * QB],
                    start=(t == 0), stop=(t == MP - 1), perf_mode=DRM)
            recip = rcpool.tile([P, QB], F32, tag="recip")
            nc.vector.reciprocal_approx_fast(recip[:], ps[:])
            cell.append(recip)
        return emit

    def av_unit(h, nb, et, cell):
        def emit():
            expT, V28 = expTs[h], V28s[h]
            ps = psAV.tile([P, QB], F32, tag="psAV")
            for t in range(MP):
                nc.tensor.matmul(
                    ps[:],
                    V28[:, 2 * t:2 * t + 2, et * P:(et + 1) * P],
                    expT[:, 2 * t:2 * t + 2, nb * QB:(nb + 1) * QB],
                    start=(t == 0), stop=(t == MP - 1), perf_mode=DRM)
            cell.append(ps)
        return emit

    def tail_unit(h, nb, cell, dma_eng=None):
        def emit():
            recip, ps0, ps1 = cell
            pair = (ps0, ps1)
            last = (h == HPC - 1)
            for et in range(DC):
                osl = out_acc[:, et, nb * QB:(nb + 1) * QB]
                if h == 0:
                    nc.vector.tensor_mul(osl, pair[et][:], recip[:])
                else:
                    sc = scpool.tile([P, QB], F32, tag="sc")
                    nc.vector.tensor_mul(sc[:], pair[et][:], recip[:])
                    ae = nc.gpsimd if et == 0 else nc.vector
                    ae.tensor_add(osl, osl, sc[:])
                if last:
                    (dma_eng or nc.sync).dma_start(
                        outd[et * P:(et + 1) * P, nb * QB:(nb + 1) * QB], osl)
        return emit

    def av_units_half(h, hf):
        units = []
        for nb in (2 * hf, 2 * hf + 1):
            cell = []
            units.append(denom_unit(h, nb, cell))
            units.append(av_unit(h, nb, 0, cell))
            units.append(av_unit(h, nb, 1, cell))
            units.append(tail_unit(h, nb, cell))
        return units

    # Pipeline fill: the first t-proj pair inline so exp(hf=0, mt=0..3) can
    # start as soon as k/q/wm land; the rest of proj(0) rides the first
    # phase's slots.
    tT8s[0] = tpool.tile([P, DC, S], FP8, tag="tT", name="tT8")
    V28s[0] = vpool.tile([P, MT, D], FP8, tag="V2", name="V28")
    for et, (pool, ptag) in enumerate([(psPD, "psPD"), (psAV, "psAV")]):
        ps0 = pool.tile([P, QB], F32, tag=ptag, name="ps0")
        for dc in range(DC):
            nc.tensor.matmul(ps0[:], weights[0][0][:, dc, et * P:(et + 1) * P],
                             kT[:, dc, 0:QB], start=(dc == 0),
                             stop=(dc == DC - 1))
        nc.vector.tensor_copy(tT8s[0][:, et, 0:P], ps0[:, 0:P])
        nc.vector.tensor_copy(tT8s[0][:, et, P:QB], ps0[:, P:QB])
    rest0 = []
    for mb in range(1, NQB):
        rest0.append(tproj_group(0, 0, mb, psPD, "psPD"))
        rest0.append(tproj_group(0, 1, mb, psAV, "psAV"))
    for mp in range(MP):
        rest0.append(vproj_group(0, mp))

    bg_av = deque(carry)
    for h in range(HPC):
        if h + 1 < HPC:
            weights[h + 1] = load_weights(h + 1)
            bg_proj = deque(proj_units(h + 1))
        else:
            bg_proj = deque()
        if h == 0:
            bg_proj = deque(rest0 + list(bg_proj))

        expT = epool.tile([P, MT, S], FP8, tag="exp", name="expT")
        expTs[h] = expT
        ku_sb = weights[h][2]
        tT8 = tT8s[h]
        chunk_ps = {}
        for hf in range(2):
            chunked = last_rep and h == HPC - 1 and hf == 1
            for mt in range(MT):
                ps = psS.tile([P, 2 * QB], F32, tag="psS")
                for j in range(2):
                    nb = 2 * hf + j
                    nc.tensor.matmul(
                        ps[:, j * QB:(j + 1) * QB],
                        tT8[:, :, mt * P:(mt + 1) * P],
                        qT8[:, :, nb * QB:(nb + 1) * QB],
                        start=True, stop=True, perf_mode=DRM)
                nc.scalar.activation(
                    expT[:, mt:mt + 1, hf * HS:(hf + 1) * HS],
                    ps[:], EXP, bias=ku_sb[:, mt:mt + 1], scale=1.0 / ST)
                if bg_av:
                    bg_av.popleft()()
                # At h=0 hold the first two slots pump-free: the mb>=1
                # t-proj units would queue in-order PE matmuls that stall
                # on not-yet-landed kT DMA, delaying the next score pair.
                if nwarm and h == 0 and hf == 0 and mt < 2:
                    npop = 0
                else:
                    slots_left = (2 - hf) * MT - mt
                    npop = -(-len(bg_proj) // slots_left) if bg_proj else 0
                for _ in range(npop):
                    bg_proj.popleft()()
                if chunked and mt >= MT - 6:
                    # pairs t=0..5 of the final-half AV accumulate now; the
                    # psAV/psPD banks of the (3,0) units are free by slot 10.
                    t = mt - (MT - 6)
                    for ci, (nb, et) in enumerate(
                            [(2, 0), (2, 1), (3, 0), (3, 1)]):
                        if t == 0:
                            pool = psAV if nb == 2 else psPD
                            chunk_ps[(nb, et)] = pool.tile(
                                [P, QB], F32, tag=pool.name if False else
                                ("psAV" if nb == 2 else "psPD"),
                                name="ps_chunk")
                        nc.tensor.matmul(
                            chunk_ps[(nb, et)][:],
                            V28s[h][:, 2 * t:2 * t + 2, et * P:(et + 1) * P],
                            expT[:, 2 * t:2 * t + 2, nb * QB:(nb + 1) * QB],
                            start=(t == 0), stop=False, perf_mode=DRM)
            for u in bg_av:
                u()
            bg_av = deque(av_units_half(h, hf))
        for u in bg_proj:
            u()

    if not last_rep:
        # Last half's AV work is returned to overlap the next iteration's
        # fill (pumped through the next iteration's slots, denominators
        # first per query block).
        u = list(bg_av)
        return [u[0], u[4], u[1], u[2], u[5], u[6], u[3], u[7]]

    # Final repeat: finish the chunked AV (pairs 6,7), then denominators
    # into a psS tile, reciprocals, and tails -- the shortest possible
    # post-exp critical path.
    expT, V28 = expTs[HPC - 1], V28s[HPC - 1]
    ps_d = psS.tile([P, 2 * QB], F32, tag="psS", name="ps_d")
    recips = {}
    for j, nb in enumerate((2, 3)):
        half = ps_d[:, j * QB:(j + 1) * QB]
        for t in range(MP):
            nc.tensor.matmul(
                half, ones8[:],
                expT[:, 2 * t:2 * t + 2, nb * QB:(nb + 1) * QB],
                start=(t == 0), stop=(t == MP - 1), perf_mode=DRM)
        recip = rcpool.tile([P, QB], F32, tag="recip")
        nc.vector.reciprocal_approx_fast(recip[:], half)
        recips[nb] = recip
    for t in (MP - 2, MP - 1):
        for nb, et in [(2, 0), (2, 1), (3, 0), (3, 1)]:
            nc.tensor.matmul(
                chunk_ps[(nb, et)][:],
                V28[:, 2 * t:2 * t + 2, et * P:(et + 1) * P],
                expT[:, 2 * t:2 * t + 2, nb * QB:(nb + 1) * QB],
                start=False, stop=(t == MP - 1), perf_mode=DRM)
    for nb in (2, 3):
        cell = [recips[nb], chunk_ps[(nb, 0)], chunk_ps[(nb, 1)]]
        tail_unit(HPC - 1, nb, cell,
                  dma_eng=(nc.sync if nb == 2 else nc.scalar))()
    return []


_progs = {}


def _get_prog(repeat=1):
    if repeat not in _progs:
        _progs[repeat] = build_program(repeat)
    return _progs[repeat]


def _prepare_in_maps(k, v, q, Wk, bk, Wv, bv, Wq, bq, Wo, bo):
    scale = np.float32(1.0 / 16.0)  # 1/sqrt(D), exact power of two
    E4 = ml_dtypes.float8_e4m3
    qt8 = [np.ascontiguousarray(q[b].T).astype(E4) for b in range(B)]
    in_maps = []
    for core in range(2 * B):
        b, g = core // 2, core % 2
        hs = list(range(g * HPC, (g + 1) * HPC))
        wm = np.stack([
            (Wk[h].astype(np.float64)
             @ (Wq[h].astype(np.float64) * scale).T).astype(np.float32)
            * np.float32(ST)
            for h in hs])
        w2 = np.stack([
            (Wv[h].astype(np.float64)
             @ Wo[h::H].astype(np.float64)).astype(np.float32)
            * np.float32(SV)
            for h in hs])
        ku = np.stack([
            (k[b].astype(np.float64)
             @ (Wk[h].astype(np.float64) @ (bq[h].astype(np.float64) * scale))
             ).astype(np.float32).reshape(MT, P).T
            for h in hs])
        in_maps.append({
            "kt": np.ascontiguousarray(k[b].T),
            "vt": np.ascontiguousarray(v[b].T),
            "qt": qt8[b],
            "wm": np.ascontiguousarray(wm),
            "w2": np.ascontiguousarray(w2),
            "ku": np.ascontiguousarray(ku),
        })
    return in_maps


def _bo_prime(bv, Wo, bo):
    acc = bo.astype(np.float64).copy()
    for h in range(H):
        acc += bv[h].astype(np.float64) @ Wo[h::H].astype(np.float64)
    return acc.astype(np.float32)


def _run_spmd(in_maps, repeat=1, **kwargs):
    nc = _get_prog(repeat)
    return run_bass_kernel_spmd(nc, in_maps, core_ids=list(range(2 * B)),
                                **kwargs)


def kernel(k, v, q, Wk, bk, Wv, bv, Wq, bq, Wo, bo):
    arrs = [np.asarray(x, dtype=np.float32)
            for x in (k, v, q, Wk, bk, Wv, bv, Wq, bq, Wo, bo)]
    k, v, q, Wk, bk, Wv, bv, Wq, bq, Wo, bo = arrs
    in_maps = _prepare_in_maps(k, v, q, Wk, bk, Wv, bv, Wq, bq, Wo, bo)
    rr = _run_spmd(in_maps)
    bop = _bo_prime(bv, Wo, bo)
    out = np.empty((B, S, D), np.float32)
    for b in range(B):
        out[b] = (rr.results[2 * b]["out"].T + rr.results[2 * b + 1]["out"].T
                  + bop)
    return out
